# revision 10
# baseline (speedup 1.0000x reference)
"""Bass/Tile Trainium2 kernel for additive (Bahdanau/'cat') attention.

Problem (per batch b):
  A[i,d]      = sum_a context[i,a] * attn_w[a,d] + attn_b[d]
  O[o,d]      = sum_e output[o,e]  * dec_w[e,d]  + dec_b[d]
  scores[o,i] = sum_d query_w[d] * tanh(A[i,d] + O[o,d])   (+query_b: softmax-invariant)
  attn        = softmax_i(scores)
  mix[o,a]    = sum_i attn[o,i] * context[i,a]
  out[o,d]    = tanh([mix | output] @ out_w + out_b)

Sharding: pure data-parallel over batch, B=8 -> one batch per NeuronCore,
weights broadcast, no collectives.

Algorithm: the naive tanh over the [out, in, dec] = 16.7M-element grid is
ACT-engine-bound (~110us at 0.83ns/elem, no fast modes on ACT). Instead we
use a separable rank-M expansion fitted offline (fit_lm.py + scores_lm.py,
the latter minimizing the first-order softmax/attn error directly):

  tanh(a + o) ~= sum_m c_m * tanh(al_m*a + s_m) * tanh(g_m*o + u_m)
                 (+ pure-o terms, dropped: constant over i => softmax-invariant)

so  scores[o,i] = sum_{m,d} F_m[d,i] * G_m[d,o]
  F_m[d,i] = tanh(al_m*A[i,d] + s_m)            (ACT, M instrs over [512,512])
  G_m[d,o] = c_m*q_d*tanh(g_m*O[o,d] + u_m)     (one batched ACT + DVE mults)

and the scores contraction is a PE matmul with K = M*512, writing
scores[o,i] straight into one PSUM bank in softmax layout. ACT work drops
~M*512/16.7M => ~4x less than baseline; everything else hides under it.

Schedule notes:
  * inputs pre-cast to bf16 on the host (layout prep, like the baseline's
    transposes). Critical tensors are split into per-chunk dma_starts
    spread across engine queues (a single dma_start's descriptors land on
    only ~2 HW queues -> ~11us for 512KB; 4 chunks in parallel ~3us), with
    contraction-outer matmul loops so the PE pipelines under DMA arrival.
  * O^T path first; batched o-side tanh on ACT while A^T runs; F_0 split
    per d-chunk pair; psum->sbuf bias adds split DVE/GPSIMD.
  * output^T half of the final projection pre-accumulated right after the
    last scores matmul (keeps PE warm across the softmax bubble); the
    softmax normalizer is applied via a rank-1-broadcast recip ROW on the
    mix^T copies, so exp -> transpose -> mix never waits on it.
  * attn written in bf16 (0.2% rms, far under budget), host converts.
"""

import numpy as np
import ml_dtypes

import concourse.bass as bass
import concourse.tile as tile
import concourse.bass_utils as bass_utils
from concourse import bacc, mybir
from concourse.masks import make_identity

B, OUT_LEN, IN_LEN, DEC, ATTN = 8, 64, 512, 512, 512
P = 128
F32 = mybir.dt.float32
BF16 = mybir.dt.bfloat16
AF = mybir.ActivationFunctionType
ALU = mybir.AluOpType

DC = DEC // P             # 4 d-chunks
AC = ATTN // P            # 4 a-chunks
IC = IN_LEN // P          # 4 i-chunks
EC = DEC // P             # 4 e-chunks
CC = (ATTN + DEC) // P    # 8 combined chunks

N_CORES = 8

# ---- separable fit constants (offline fit; see fit_lm.py / scores_lm.py) ----
# tanh(a+o) ~= sum_m FIT_C[m] * tanh(FIT_AL[m]*a + FIT_S[m]) * tanh(FIT_G[m]*o + FIT_U[m])
FIT_C = [0.63188261, -1.8903085, -3.8538487, -2.2637175, 2.2426041, -2.1557358, -1.992945, -2.6049868, -0.23359226, 4.2867793, -1.5083507, -2.7642975]
FIT_AL = [1.3092893, 1.3259381, 0.049854606, 1.2805608, 1.2299614, 1.3389784, -1.3341627, 1.116251, 1.6377377, 1.0565056, 1.4606415, 1.256167]
FIT_S = [-1.2789462, -2.7136524, 0.066814237, 0.41511206, -0.46557431, 0.2425577, 2.7317492, -0.62933844, 4.0571026, 1.2939587, 2.2494203, 1.1480287]
FIT_G = [1.5285393, 1.2258485, 0.21961189, -1.4995473, 1.6804221, 1.5382604, 0.8424759, 1.5488452, -0.70002981, 1.2579121, 1.0751596, 1.4162105]
FIT_U = [2.2931631, 1.8828184, 0.0025254345, 0.43584522, 1.0834487, -0.56605694, 1.5279011, 0.94040163, -2.4601071, -1.8167801, -1.9738368, -2.0085016]
M = len(FIT_C)


def _build_body(tc):
    nc = tc.nc

    # ---- DRAM I/O (per-core shard shapes) ----
    ctx_t_d = nc.dram_tensor("ctx_t", [ATTN, IN_LEN], BF16, kind="ExternalInput").ap()
    ctx_d = nc.dram_tensor("ctx", [IN_LEN, ATTN], BF16, kind="ExternalInput").ap()
    attn_w_d = nc.dram_tensor("attn_w", [ATTN, DEC], BF16, kind="ExternalInput").ap()
    dec_w_d = nc.dram_tensor("dec_w", [DEC, DEC], BF16, kind="ExternalInput").ap()
    output_t_d = nc.dram_tensor("output_t", [DEC, OUT_LEN], BF16, kind="ExternalInput").ap()
    out_w_d = nc.dram_tensor("out_w", [ATTN + DEC, DEC], BF16, kind="ExternalInput").ap()
    attn_b_d = nc.dram_tensor("attn_b", [ATTN, 1], F32, kind="ExternalInput").ap()
    dec_b_d = nc.dram_tensor("dec_b", [DEC, 1], F32, kind="ExternalInput").ap()
    query_w_d = nc.dram_tensor("query_w", [DEC, 1], F32, kind="ExternalInput").ap()
    out_b_d = nc.dram_tensor("out_b", [1, DEC], F32, kind="ExternalInput").ap()
    out_d = nc.dram_tensor("out", [OUT_LEN, DEC], F32, kind="ExternalOutput").ap()
    attn_d = nc.dram_tensor("attn", [OUT_LEN, IN_LEN], BF16, kind="ExternalOutput").ap()

    from contextlib import ExitStack

    with ExitStack() as ctx:
        const = ctx.enter_context(tc.tile_pool(name="const", bufs=1))
        statics = ctx.enter_context(tc.tile_pool(name="statics", bufs=1))
        fpool = ctx.enter_context(tc.tile_pool(name="fpool", bufs=3))
        psum = ctx.enter_context(tc.tile_pool(name="psum", bufs=2, space="PSUM"))

        dec_w_bf = statics.tile([P, EC, DEC], BF16)
        outT_bf = statics.tile([P, EC, OUT_LEN], BF16)
        attn_w_bf = statics.tile([P, AC, DEC], BF16)
        ctxT_bf = statics.tile([P, AC, IN_LEN], BF16)
        ctx_bf = statics.tile([P, IC, ATTN], BF16)
        out_w_bf = statics.tile([P, CC, DEC], BF16)

        # -------- input DMAs: chunked + spread across engine queues -------
        # sync: the A^T path (ctxT + attn_w interleaved)
        for ac in range(AC):
            nc.sync.dma_start(ctxT_bf[:, ac, :], ctx_t_d[ac * P : (ac + 1) * P, :])
            nc.sync.dma_start(attn_w_bf[:, ac, :], attn_w_d[ac * P : (ac + 1) * P, :])
        # scalar: O^T path + small consts
        nc.scalar.dma_start(outT_bf[:], output_t_d.rearrange("(ec p) o -> p ec o", p=P))
        nc.scalar.dma_start(dec_w_bf[:, 0, :], dec_w_d[0:P, :])
        nc.scalar.dma_start(dec_w_bf[:, 1, :], dec_w_d[P : 2 * P, :])
        qw_f = const.tile([P, DC], F32)
        attn_bias = const.tile([P, DC], F32)
        dec_bias = const.tile([P, DC], F32)
        outb_row_f = const.tile([1, DEC], F32)
        nc.scalar.dma_start(qw_f[:], query_w_d.rearrange("(dc p) one -> p dc one", p=P))
        nc.scalar.dma_start(dec_bias[:], dec_b_d.rearrange("(dc p) one -> p dc one", p=P))
        nc.scalar.dma_start(attn_bias[:], attn_b_d.rearrange("(dc p) one -> p dc one", p=P))
        nc.scalar.dma_start(outb_row_f[:], out_b_d)

        # ---------------- constants ----------------
        ident = const.tile([P, P], F32)
        make_identity(nc, ident)

        # gpsimd: remaining dec_w chunks + mid-kernel tensors
        nc.gpsimd.dma_start(dec_w_bf[:, 2, :], dec_w_d[2 * P : 3 * P, :])
        nc.gpsimd.dma_start(dec_w_bf[:, 3, :], dec_w_d[3 * P : 4 * P, :])
        nc.gpsimd.dma_start(ctx_bf[:, 0:2], ctx_d.rearrange("(ic p) a -> p ic a", p=P, ic=4)[:, 0:2])
        nc.gpsimd.dma_start(ctx_bf[:, 2:4], ctx_d.rearrange("(ic p) a -> p ic a", p=P, ic=4)[:, 2:4])
        nc.gpsimd.dma_start(out_w_bf[:, 0:4], out_w_d.rearrange("(cc p) d -> p cc d", p=P, cc=8)[:, 0:4])
        nc.gpsimd.dma_start(out_w_bf[:, 4:8], out_w_d.rearrange("(cc p) d -> p cc d", p=P, cc=8)[:, 4:8])

        ident_bf = const.tile([P, P], BF16)
        nc.vector.tensor_copy(ident_bf[:], ident[:])

        # HAM warmup: real matmul activity ramps the PE clock gate to full
        # speed before the real matmuls arrive.
        wu = psum.tile([P, P], F32, tag="mm", bufs=3)
        for _ in range(20):
            nc.tensor.matmul(wu[:], ident_bf[:], ident_bf[:], start=True, stop=True)

        outb_row_bf = const.tile([1, DEC], BF16)
        nc.vector.tensor_copy(outb_row_bf[:], outb_row_f[:])
        ones_row = const.tile([1, OUT_LEN], BF16)
        nc.vector.memset(ones_row[:], 1.0)
        ones_rowP = const.tile([1, P], BF16)
        nc.vector.memset(ones_rowP[:], 1.0)
        ones64 = const.tile([P, OUT_LEN], BF16)
        nc.vector.memset(ones64[:], 1.0)
        # per-m F-side bias tiles (activation bias must be an SBUF AP)
        sbias = const.tile([P, M], F32)
        for m in range(M):
            nc.vector.memset(sbias[:, m : m + 1], float(FIT_S[m]))

        # Qbc[p, dc, o] = q_d (broadcast over o); QC_m = c_m * Qbc
        Qbc = const.tile([P, DC, OUT_LEN], BF16)
        for dc in range(DC):
            nc.vector.tensor_scalar_mul(Qbc[:, dc, :], ones64[:], qw_f[:, dc : dc + 1])
        QC = const.tile([P, M, DC, OUT_LEN], BF16)
        for m in range(M):
            nc.vector.tensor_scalar_mul(QC[:, m], Qbc[:], float(FIT_C[m]))

        # ---------------- O^T [d, o] + bias (ec-outer: pipelines under DMA)
        OTb = statics.tile([P, DC, OUT_LEN], BF16)
        po = [psum.tile([P, OUT_LEN], F32, tag="at", bufs=4, name=f"po_{dc}")
              for dc in range(DC)]
        for ec in range(EC):
            for dc in range(DC):
                nc.tensor.matmul(
                    po[dc][:],
                    dec_w_bf[:, ec, dc * P : (dc + 1) * P],
                    outT_bf[:, ec, :],
                    start=(ec == 0),
                    stop=(ec == EC - 1),
                )
        for dc in range(DC):
            nc.vector.tensor_scalar_add(OTb[:, dc, :], po[dc][:], dec_bias[:, dc : dc + 1])

        # o-side: TIN[:, m] = g_m*OTb + u_m ; TAU = tanh(TIN) (one ACT instr)
        TIN = statics.tile([P, M, DC, OUT_LEN], BF16)
        for m in range(M):
            nc.vector.tensor_scalar(
                TIN[:, m], OTb[:], float(FIT_G[m]), float(FIT_U[m]),
                op0=ALU.mult, op1=ALU.add,
            )
        TAU = statics.tile([P, M, DC, OUT_LEN], BF16)
        nc.scalar.activation(TAU[:], TIN[:], AF.Tanh)

        # ---------------- A^T [d, i] + bias (ac-outer) ----------------
        ATb = statics.tile([P, DC, IN_LEN], BF16)
        pa = [psum.tile([P, IN_LEN], F32, tag="at", bufs=4, name=f"pa_{dc}")
              for dc in range(DC)]
        for ac in range(AC):
            for dc in range(DC):
                nc.tensor.matmul(
                    pa[dc][:],
                    attn_w_bf[:, ac, dc * P : (dc + 1) * P],
                    ctxT_bf[:, ac, :],
                    start=(ac == 0),
                    stop=(ac == AC - 1),
                )
        # psum -> bf16 + bias (GPSIMD cannot read PSUM, so all on DVE)
        for dc in range(DC):
            nc.vector.tensor_scalar_add(ATb[:, dc, :], pa[dc][:], attn_bias[:, dc : dc + 1])

        # G = TAU (.) QC (after the ATb adds in DVE order so F_0 isn't gated)
        G = statics.tile([P, M, DC, OUT_LEN], BF16)
        nc.vector.tensor_tensor(G[:, 0 : M // 2], TAU[:, 0 : M // 2], QC[:, 0 : M // 2], op=ALU.mult)
        nc.vector.tensor_tensor(G[:, M // 2 : M], TAU[:, M // 2 : M], QC[:, M // 2 : M], op=ALU.mult)

        # ---------------- main stream: F_m + scores matmul ----------------
        scores_ps = psum.tile([OUT_LEN, IN_LEN], F32, tag="sc", bufs=1, name="scores")
        for m in range(M):
            F = fpool.tile([P, DC, IN_LEN], BF16, tag="F", name=f"F_{m}")
            if m == 0:
                # split: start tanh as soon as the first two ATb chunks land
                nc.scalar.activation(
                    F[:, 0:2], ATb[:, 0:2], AF.Tanh,
                    bias=sbias[:, m : m + 1], scale=float(FIT_AL[m]),
                )
                nc.scalar.activation(
                    F[:, 2:DC], ATb[:, 2:DC], AF.Tanh,
                    bias=sbias[:, m : m + 1], scale=float(FIT_AL[m]),
                )
            else:
                nc.scalar.activation(
                    F[:], ATb[:], AF.Tanh,
                    bias=sbias[:, m : m + 1], scale=float(FIT_AL[m]),
                )
            for dc in range(DC):
                nc.tensor.matmul(
                    scores_ps[:],
                    G[:, m, dc, :],
                    F[:, dc, :],
                    start=(m == 0 and dc == 0),
                    stop=(m == M - 1 and dc == DC - 1),
                )

        # final projection: pre-accumulate the output^T half right after the
        # last scores matmul — fills the softmax bubble and keeps PE warm.
        pf = psum.tile([OUT_LEN, DEC], F32, tag="at", bufs=4, name="pf")
        for k, cc in enumerate(range(AC, CC)):
            nc.tensor.matmul(
                pf[:], outT_bf[:, cc - AC, :], out_w_bf[:, cc, :],
                start=(k == 0), stop=False,
            )
        nc.tensor.matmul(pf[:], ones_row[:], outb_row_bf[:], start=False, stop=False)

        # ---------------- softmax ----------------
        exp_bf = statics.tile([OUT_LEN, IN_LEN], BF16)
        sums = statics.tile([OUT_LEN, 1], F32)
        recip = statics.tile([OUT_LEN, 1], F32)
        recip_bf = statics.tile([OUT_LEN, 1], BF16)
        attn_bf = statics.tile([OUT_LEN, IN_LEN], BF16)
        nc.scalar.activation(exp_bf[:], scores_ps[:], AF.Exp, accum_out=sums[:])
        nc.vector.reciprocal(recip[:], sums[:])
        nc.vector.tensor_copy(recip_bf[:], recip[:])
        # attn output path (off the mix critical path)
        nc.vector.tensor_scalar_mul(attn_bf[:], exp_bf[:], recip[:])
        nc.sync.dma_start(attn_d[:], attn_bf[:])

        # RECIP_ROW[p, o] = 1/sums[o] broadcast across partitions:
        # transpose recip to a row, then rank-1 with a ones column.
        rr_ps = psum.tile([1, OUT_LEN], BF16, tag="mm", bufs=3, name="rrT")
        nc.tensor.transpose(rr_ps[:], recip_bf[:], ident_bf[0:OUT_LEN, 0:OUT_LEN])
        rrow_bf = statics.tile([1, OUT_LEN], BF16)
        nc.vector.tensor_copy(rrow_bf[:], rr_ps[:])
        rb_ps = psum.tile([P, OUT_LEN], F32, tag="mm", bufs=3, name="rbc")
        nc.tensor.matmul(rb_ps[:], ones_rowP[:], rrow_bf[:], start=True, stop=True)
        RECIP_ROW = statics.tile([P, OUT_LEN], BF16)
        nc.vector.tensor_copy(RECIP_ROW[:], rb_ps[:])

        # exp^T via PE transposes (normalizer NOT needed here)
        expT_bf = statics.tile([P, IC, OUT_LEN], BF16)
        for ic in range(IC):
            pt = psum.tile([P, OUT_LEN], BF16, tag="mm", bufs=3, name=f"pt_{ic}")
            nc.tensor.transpose(
                pt[:], exp_bf[:, ic * P : (ic + 1) * P], ident_bf[0:OUT_LEN, 0:OUT_LEN]
            )
            nc.vector.tensor_copy(expT_bf[:, ic, :], pt[:])

        # unnormalized mix^T chunks [a, o]; normalize on the psum->sbuf copy
        combT_bf = statics.tile([P, AC, OUT_LEN], BF16)
        for ac in range(AC):
            pm = psum.tile([P, OUT_LEN], F32, tag="mm", bufs=3, name=f"pm_{ac}")
            for ic in range(IC):
                nc.tensor.matmul(
                    pm[:],
                    ctx_bf[:, ic, ac * P : (ac + 1) * P],
                    expT_bf[:, ic, :],
                    start=(ic == 0),
                    stop=(ic == IC - 1),
                )
            nc.vector.tensor_tensor(combT_bf[:, ac, :], pm[:], RECIP_ROW[:], op=ALU.mult)

        # remaining (mix) half of the projection
        for cc in range(AC):
            nc.tensor.matmul(
                pf[:], combT_bf[:, cc, :], out_w_bf[:, cc, :],
                start=False, stop=(cc == AC - 1),
            )
        out_sb = statics.tile([OUT_LEN, DEC], F32)
        nc.scalar.activation(out_sb[:], pf[:], AF.Tanh)
        nc.sync.dma_start(out_d[:], out_sb[:])


_CACHE = {}


def build_nc():
    if "nc" in _CACHE:
        return _CACHE["nc"]
    nc = bacc.Bacc(
        "TRN2",
        target_bir_lowering=False,
        debug=False,
        num_devices=N_CORES,
    )
    with tile.TileContext(nc) as tc:
        _build_body(tc)
    nc.compile()
    _CACHE["nc"] = nc
    return nc


def kernel(**inputs):
    nc = build_nc()

    bf = ml_dtypes.bfloat16
    f32 = lambda k: np.ascontiguousarray(np.asarray(inputs[k], dtype=np.float32))
    output = f32("output")
    context = f32("context")
    shared = {
        "attn_w": np.ascontiguousarray(f32("attn_w").astype(bf)),
        "dec_w": np.ascontiguousarray(f32("dec_w").astype(bf)),
        "out_w": np.ascontiguousarray(f32("out_w").astype(bf)),
        "attn_b": f32("attn_b").reshape(ATTN, 1),
        "dec_b": f32("dec_b").reshape(DEC, 1),
        "query_w": f32("query_w").reshape(DEC, 1),
        "out_b": f32("out_b").reshape(1, DEC),
    }
    in_maps = []
    for b in range(N_CORES):
        m = dict(shared)
        m["ctx"] = np.ascontiguousarray(context[b].astype(bf))
        m["ctx_t"] = np.ascontiguousarray(context[b].T.astype(bf))
        m["output_t"] = np.ascontiguousarray(output[b].T.astype(bf))
        in_maps.append(m)

    res = bass_utils.run_bass_kernel_spmd(nc, in_maps, core_ids=list(range(N_CORES)))
    _CACHE["last_results"] = res
    out = np.stack([res.results[b]["out"] for b in range(N_CORES)])
    attn = np.stack(
        [res.results[b]["attn"].astype(np.float32) for b in range(N_CORES)]
    )
    return out, attn
